# revision 15
# baseline (speedup 1.0000x reference)
"""Trainium2 Bass kernel for nn_CrossAttentionLayer (B=16, F=512, D=768, H=12).

Sharding: data-parallel over batch; each of the 8 cores handles 2 batches
end-to-end (no collectives).

Device-side layout strategy (per core):
- Everything keeps the contraction dim on SBUF partitions:
  * q/k projections are produced transposed: qT/kT [d_feat, tok]
  * v projections natural [tok, d_feat], augmented with a ones column per
    head so the unnormalized AV matmul also yields softmax denominators.
  * scoresT [key, tok_q] -> ACT exp (scale=1/8 folded) -> bf16 expT
  * AV: UT[65, q] = [v_h | 1]^T @ expT_h  (row 64 = softmax denominator)
  * normalization deferred: attT and mean-weight tiles are scaled by
    broadcast reciprocal tiles built with K=1/K=2 matmuls.
  * head-mean of attention weights: PE identity-matmul accumulation over
    heads in PSUM (output stays transposed; host untransposes).
  * out-proj: fusedT[feat, q] = W_aug^T-slices @ attT, bias via rank-1
    matmul with a host-appended bias row; PSUM DMA'd straight to DRAM.
Host does all pre/post transposes and the 8-way shard/gather.
"""

import math

import numpy as np

_B, _F, _D, _H, _DK = 16, 512, 768, 12, 64
_NB = 2  # batches per core
_NCORES = 8
_KT = _D // 128  # 6 contraction tiles over D
_TT = _F // 128  # 4 tok tiles

_cache = {}


def _build():
    import concourse.bacc as bacc
    import concourse.mybir as mybir
    import concourse.tile as tile
    from concourse.masks import make_identity

    F32 = mybir.dt.float32
    F32R = mybir.dt.float32r
    BF16 = mybir.dt.bfloat16
    EXP = mybir.ActivationFunctionType.Exp

    nc = bacc.Bacc(None, target_bir_lowering=False)

    # ---- DRAM I/O (per core) ----
    rowT = nc.dram_tensor("rowT", [_NB, _D, _F], F32R, kind="ExternalInput")
    colT = nc.dram_tensor("colT", [_D, _F], F32R, kind="ExternalInput")
    wts = {
        name: nc.dram_tensor(name, [_D, _D], F32R, kind="ExternalInput")
        for name in ["wt_rc_q", "wt_rc_k", "wt_rc_v", "wt_cr_q", "wt_cr_k", "wt_cr_v"]
    }
    w_row_aug = nc.dram_tensor("w_row_aug", [_D + 1, _D], F32R, kind="ExternalInput")
    w_col_aug = nc.dram_tensor("w_col_aug", [_D + 1, _D], F32R, kind="ExternalInput")

    fusedT_row = nc.dram_tensor("fusedT_row", [_NB, _D, _F], F32, kind="ExternalOutput")
    fusedT_col = nc.dram_tensor("fusedT_col", [_NB, _D, _F], F32, kind="ExternalOutput")
    meanwT_r2c = nc.dram_tensor("meanwT_r2c", [_NB, _F, _F], F32, kind="ExternalOutput")
    meanwT_c2r = nc.dram_tensor("meanwT_c2r", [_NB, _F, _F], F32, kind="ExternalOutput")

    from contextlib import ExitStack

    with tile.TileContext(nc) as tc, ExitStack() as stack:
        consts = stack.enter_context(tc.tile_pool(name="consts", bufs=1))
        persist = stack.enter_context(tc.tile_pool(name="persist", bufs=1))

        # pair-indicator matrices for broadcast matmuls (via inline consts:
        # engine writes must start at a 32-aligned partition)
        ind2_np = np.zeros((2, 128), np.float32)
        ind2_np[0, 0:64] = 1.0
        ind2_np[1, 64:128] = 1.0
        ind_e_np = np.zeros((2, 128), np.float32)
        ind_e_np[0, :] = 1.0
        ind_o_np = np.zeros((2, 128), np.float32)
        ind_o_np[1, :] = 1.0
        ind2 = consts.tile([2, 128], F32R)
        ind2_dram = nc.inline_tensor(ind2_np, name="ind2_dram")
        nc.sync.dma_start(out=ind2, in_=ind2_dram[:, :].bitcast(F32R))
        ind_e = consts.tile([2, 128], F32R)
        ind_e_dram = nc.inline_tensor(ind_e_np, name="ind_e_dram")
        nc.sync.dma_start(out=ind_e, in_=ind_e_dram[:, :].bitcast(F32R))
        ind_o = consts.tile([2, 128], F32R)
        ind_o_dram = nc.inline_tensor(ind_o_np, name="ind_o_dram")
        nc.sync.dma_start(out=ind_o, in_=ind_o_dram[:, :].bitcast(F32R))
        ident = consts.tile([128, 128], BF16)
        make_identity(nc, ident)

        # persistent activations (bf16)
        cqT = persist.tile([128, _KT * _F], BF16, tag="cqT")
        ckT = persist.tile([128, _KT * _F], BF16, tag="ckT")
        cv_aug = persist.tile([128, _TT, _H * 65], BF16, tag="cv_aug")
        rqT = [persist.tile([128, _KT * _F], BF16, tag=f"rqT{b}", name=f"rqT{b}") for b in range(_NB)]
        rkT = [persist.tile([128, _KT * _F], BF16, tag=f"rkT{b}", name=f"rkT{b}") for b in range(_NB)]
        rv_aug = [
            persist.tile([128, _TT, _H * 65], BF16, tag=f"rv_aug{b}", name=f"rv_aug{b}") for b in range(_NB)
        ]

        # ---------- Phase A/B: projections (QKV weights scoped) ----------
        def proj_T(dst, w_sb, x_sb, pp, cpool):
            # dst[feat_tile ot] [128, F] = (W^T)^T-slice @ xT  (feats on partitions)
            for ot in range(_KT):
                ps = pp.tile([128, _F], F32, tag="projT_ps")
                for kt in range(_KT):
                    nc.tensor.matmul(
                        ps,
                        w_sb[:, kt, ot * 128 : (ot + 1) * 128],
                        x_sb[:, kt * _F : (kt + 1) * _F],
                        start=(kt == 0),
                        stop=(kt == _KT - 1),
                    )
                nc.vector.tensor_copy(dst[:, ot * _F : (ot + 1) * _F], ps)

        def proj_nat_aug(dst, w_sb, x_sb, pp, cpool):
            # dst[:, tt, h*65:h*65+64] = (x @ W^T) slice; col h*65+64 = 1.0
            for tt in range(_TT):
                ps = pp.tile([128, _D], F32, tag="projN_ps")
                for nh, (n0, n1) in enumerate([(0, 512), (512, 768)]):
                    for kt in range(_KT):
                        nc.tensor.matmul(
                            ps[:, n0:n1],
                            x_sb[:, kt * _F + tt * 128 : kt * _F + (tt + 1) * 128],
                            w_sb[:, kt, n0:n1],
                            start=(kt == 0),
                            stop=(kt == _KT - 1),
                        )
                ps_h = ps.rearrange("p (h d) -> p h d", h=_H)
                dst_h = dst[:, tt, :].rearrange("p (h e) -> p h e", e=65)
                nc.vector.tensor_copy(dst_h[:, :, 0:64], ps_h)
                nc.vector.memset(dst_h[:, :, 64:65], 1.0)

        with (
            tc.tile_pool(name="wqkv", bufs=2) as wp,
            tc.tile_pool(name="xin", bufs=1) as xp,
            tc.tile_pool(name="projps", bufs=2, space="PSUM") as pp,
            tc.tile_pool(name="projcp", bufs=2) as cpool,
        ):
            colT_sb = xp.tile([128, _KT * _F], F32R, tag="colT")
            nc.sync.dma_start(
                out=colT_sb.rearrange("p (t f) -> p t f", t=_KT),
                in_=colT[:, :].rearrange("(t p) f -> p t f", p=128),
            )
            rowT_sb = [
                xp.tile([128, _KT * _F], F32R, tag=f"rowT{b}", name=f"rowT{b}")
                for b in range(_NB)
            ]
            for b in range(_NB):
                nc.sync.dma_start(
                    out=rowT_sb[b].rearrange("p (t f) -> p t f", t=_KT),
                    in_=rowT[b, :, :].rearrange("(t p) f -> p t f", p=128),
                )

            # each QKV weight has exactly one consumer group: stream them
            jobs = [
                ("wt_rc_k", "T", [(ckT, colT_sb)]),
                ("wt_rc_v", "N", [(cv_aug, colT_sb)]),
                ("wt_cr_q", "T", [(cqT, colT_sb)]),
                ("wt_rc_q", "T", [(rqT[b], rowT_sb[b]) for b in range(_NB)]),
                ("wt_cr_k", "T", [(rkT[b], rowT_sb[b]) for b in range(_NB)]),
                ("wt_cr_v", "N", [(rv_aug[b], rowT_sb[b]) for b in range(_NB)]),
            ]
            for wname, kind, uses in jobs:
                w_sb = wp.tile([128, _KT, _D], F32R, tag="w_sb", name=f"sb_{wname}")
                nc.sync.dma_start(
                    out=w_sb,
                    in_=wts[wname][:, :].rearrange("(t p) o -> p t o", p=128),
                )
                for dst, x_sb in uses:
                    if kind == "T":
                        proj_T(dst, w_sb, x_sb, pp, cpool)
                    else:
                        proj_nat_aug(dst, w_sb, x_sb, pp, cpool)

        # out-proj weights, resident after QKV released
        wout_sb = {}
        wbias_sb = {}
        for name, dram in [("row", w_row_aug), ("col", w_col_aug)]:
            wout_sb[name] = persist.tile([128, _KT, _D], F32R, tag=f"wout_{name}", name=f"wout_{name}")
            nc.sync.dma_start(
                out=wout_sb[name],
                in_=dram[0:_D, :].rearrange("(t p) o -> p t o", p=128),
            )
            wbias_sb[name] = persist.tile([128, _KT], F32, tag=f"wbias_{name}", name=f"wbias_{name}")
            nc.sync.dma_start(
                out=wbias_sb[name],
                in_=dram[_D, :].rearrange("(t p) -> p t", p=128).bitcast(F32),
            )

        # ---------- Phase C: attention per (batch, direction) ----------
        with (
            tc.tile_pool(name="expp", bufs=1) as expp,
            tc.tile_pool(name="attp", bufs=1) as attp,
            tc.tile_pool(name="rsp", bufs=1) as rsp,
        ):
            for b in range(_NB):
                for attn in ("r2c", "c2r"):
                    if attn == "r2c":
                        qT, kT, v_aug = rqT[b], ckT, cv_aug
                        wout, wbias = wout_sb["row"], wbias_sb["row"]
                        fused_dram, mw_dram = fusedT_row, meanwT_r2c
                    else:
                        qT, kT, v_aug = cqT, rkT[b], rv_aug[b]
                        wout, wbias = wout_sb["col"], wbias_sb["col"]
                        fused_dram, mw_dram = fusedT_col, meanwT_c2r

                    expT = expp.tile([128, _H, _TT * _F], BF16, tag="expT")
                    attT = attp.tile([128, _KT * _F], F32R, tag="attT")
                    rs12 = rsp.tile([_H, _F], F32, tag="rs12")
                    recipA = rsp.tile([_H, _F], F32R, tag="recipA")
                    # matmul-rhs paired layout (base partition must be 0):
                    # recipA2[j, p, :] = 1/rowsum of head 2p+j
                    recipA2 = rsp.tile([2, _H // 2, _F], F32R, tag="recipA2")

                    # --- C.1: scores -> exp -> AV per head ---
                    with (
                        tc.tile_pool(name="scps", bufs=2, space="PSUM") as scps,
                        tc.tile_pool(name="utps", bufs=2, space="PSUM") as utps,
                        tc.tile_pool(name="stp", bufs=3) as stp,
                    ):
                        for h in range(_H):
                            p, half = h // 2, h % 2
                            qs = qT[
                                64 * half : 64 * half + 64, p * _F : (p + 1) * _F
                            ]
                            for chunk in range(2):
                                sc = scps.tile([128, 2 * _F], F32, tag="sc")
                                for i in range(2):
                                    kt = 2 * chunk + i
                                    ks = kT[
                                        64 * half : 64 * half + 64,
                                        p * _F + kt * 128 : p * _F + (kt + 1) * 128,
                                    ]
                                    nc.tensor.matmul(
                                        sc[:, i * _F : (i + 1) * _F],
                                        ks,
                                        qs,
                                        start=True,
                                        stop=True,
                                    )
                                nc.scalar.activation(
                                    expT[:, h, chunk * 2 * _F : (chunk + 1) * 2 * _F],
                                    sc,
                                    EXP,
                                    scale=0.125,
                                )
                            ut = utps.tile([65, _F], F32, tag="ut")
                            for kt in range(_TT):
                                nc.tensor.matmul(
                                    ut,
                                    v_aug[:, kt, h * 65 : (h + 1) * 65],
                                    expT[:, h, kt * _F : (kt + 1) * _F],
                                    start=(kt == 0),
                                    stop=(kt == _TT - 1),
                                )
                            # evacuate UT (PSUM->SBUF needs an engine; the
                            # rowsum row then moves to partition h via DMA)
                            st = stp.tile([65, _F], F32, tag="st")
                            nc.vector.tensor_copy(st, ut)
                            nc.sync.dma_start(
                                out=attT[
                                    64 * half : 64 * half + 64, p * _F : (p + 1) * _F
                                ],
                                in_=st[0:64, :].bitcast(F32R),
                            )
                            sig = (h % 2) * (_H // 2) + h // 2
                            nc.sync.dma_start(
                                out=rs12[sig : sig + 1, :], in_=st[64:65, :]
                            )

                    # --- C.2: reciprocals, normalize, mean weights ---
                    with nc.allow_low_precision(reason="f32r rounding of softmax recip"):
                        nc.vector.reciprocal(recipA, rs12)
                    # move rows into the paired matmul-rhs layout; per-row DMAs
                    # (partition-dim reshapes are not expressible as one AP)
                    for h in range(_H):
                        sig = (h % 2) * (_H // 2) + h // 2
                        nc.sync.dma_start(
                            out=recipA2[h % 2 : h % 2 + 1, h // 2, :],
                            in_=recipA[sig : sig + 1, :],
                        )

                    with (
                        tc.tile_pool(name="bcps", bufs=2, space="PSUM") as bcps,
                        tc.tile_pool(name="mwps", bufs=2, space="PSUM") as mwps,
                        tc.tile_pool(name="bcsb", bufs=2) as bcsb,
                    ):
                        for p in range(_H // 2):
                            bcA = bcps.tile([128, _F], F32, tag="bcA")
                            nc.tensor.matmul(
                                bcA, ind2, recipA2[:, p, :],
                                start=True, stop=True,
                            )
                            nc.vector.tensor_mul(
                                attT[:, p * _F : (p + 1) * _F],
                                attT[:, p * _F : (p + 1) * _F],
                                bcA,
                            )
                        for h in range(_H):
                            bcB = bcps.tile([128, _F], F32, tag="bcB")
                            nc.tensor.matmul(
                                bcB,
                                ind_e if h % 2 == 0 else ind_o,
                                recipA2[:, h // 2, :],
                                start=True, stop=True,
                            )
                            bcB_sb = bcsb.tile([128, _F], BF16, tag="bcB_sb")
                            nc.vector.tensor_copy(bcB_sb, bcB)
                            for kt in range(_TT):
                                sl = expT[:, h, kt * _F : (kt + 1) * _F]
                                nc.vector.tensor_mul(sl, sl, bcB_sb)
                        for kt in range(_TT):
                            mw = mwps.tile([128, _F], F32, tag="mw")
                            for h in range(_H):
                                nc.tensor.matmul(
                                    mw,
                                    ident,
                                    expT[:, h, kt * _F : (kt + 1) * _F],
                                    start=(h == 0),
                                    stop=(h == _H - 1),
                                )
                            mw_sb = bcsb.tile([128, _F], F32, tag="mw_sb")
                            nc.scalar.copy(mw_sb, mw)
                            nc.sync.dma_start(
                                out=mw_dram[b, kt * 128 : (kt + 1) * 128, :], in_=mw_sb
                            )

                    # --- C.3: output projection (+bias via ones row) ---
                    with (
                        tc.tile_pool(name="opps", bufs=3, space="PSUM") as opps,
                        tc.tile_pool(name="opsb", bufs=3) as opsb,
                    ):
                        for ft in range(_KT):
                            op = opps.tile([128, _F], F32, tag="op")
                            for kt in range(_KT):
                                nc.tensor.matmul(
                                    op,
                                    wout[:, kt, ft * 128 : (ft + 1) * 128],
                                    attT[:, kt * _F : (kt + 1) * _F],
                                    start=(kt == 0),
                                    stop=(kt == _KT - 1),
                                )
                            op_sb = opsb.tile([128, _F], F32, tag="op_sb")
                            nc.scalar.add(op_sb, op, wbias[:, ft : ft + 1])
                            nc.sync.dma_start(
                                out=fused_dram[b, ft * 128 : (ft + 1) * 128, :],
                                in_=op_sb,
                            )

    nc.compile()
    return nc


def _get_module():
    if "nc" not in _cache:
        _cache["nc"] = _build()
    return _cache["nc"]


def _numpy_fallback(row_embeddings, column_embeddings, row_attention_mask,
                    column_attention_mask, W_rc_q, W_rc_k, W_rc_v,
                    W_cr_q, W_cr_k, W_cr_v, W_row_out, b_row_out,
                    W_col_out, b_col_out):
    B, F, D = row_embeddings.shape
    H, DK = _H, _DK
    scale = math.sqrt(DK)
    col = np.broadcast_to(column_embeddings[None], (B, F, D))

    def heads(x):
        return x.reshape(B, F, H, DK).transpose(0, 2, 1, 3)

    def attend(q, k, v, mask):
        s = np.einsum("bhqd,bhkd->bhqk", q, k) / scale
        s = np.where(mask[:, None, :, :], s, -np.inf)
        s = s - np.nanmax(np.where(np.isinf(s), np.nan, s), axis=-1, keepdims=True)
        e = np.exp(s)
        denom = e.sum(-1, keepdims=True)
        w = np.where(denom > 0, e / np.maximum(denom, 1e-30), 0.0)
        return np.einsum("bhqk,bhkd->bhqd", w, v), w

    row_q = heads(row_embeddings @ W_rc_q.T)
    col_k = heads(col @ W_rc_k.T)
    col_v = heads(col @ W_rc_v.T)
    m1 = row_attention_mask[:, :, None] & column_attention_mask[None, None, :]
    ra, w1 = attend(row_q, col_k, col_v, m1)
    ra = ra.transpose(0, 2, 1, 3).reshape(B, F, D)
    col_q = heads(col @ W_cr_q.T)
    row_k = heads(row_embeddings @ W_cr_k.T)
    row_v = heads(row_embeddings @ W_cr_v.T)
    m2 = column_attention_mask[None, None, :] & row_attention_mask[:, :, None]
    ca, w2 = attend(col_q, row_k, row_v, m2)
    ca = ca.transpose(0, 2, 1, 3).reshape(B, F, D)
    fr = ra @ W_row_out.T + b_row_out
    fc = ca @ W_col_out.T + b_col_out
    return (fr.astype(np.float32), fc.astype(np.float32),
            w1.mean(axis=1).astype(np.float32), w2.mean(axis=1).astype(np.float32))


def kernel(row_embeddings, column_embeddings, row_attention_mask,
           column_attention_mask, W_rc_q, W_rc_k, W_rc_v,
           W_cr_q, W_cr_k, W_cr_v, W_row_out, b_row_out,
           W_col_out, b_col_out):
    args = dict(
        row_embeddings=np.asarray(row_embeddings, dtype=np.float32),
        column_embeddings=np.asarray(column_embeddings, dtype=np.float32),
        row_attention_mask=np.asarray(row_attention_mask),
        column_attention_mask=np.asarray(column_attention_mask),
        W_rc_q=np.asarray(W_rc_q, dtype=np.float32),
        W_rc_k=np.asarray(W_rc_k, dtype=np.float32),
        W_rc_v=np.asarray(W_rc_v, dtype=np.float32),
        W_cr_q=np.asarray(W_cr_q, dtype=np.float32),
        W_cr_k=np.asarray(W_cr_k, dtype=np.float32),
        W_cr_v=np.asarray(W_cr_v, dtype=np.float32),
        W_row_out=np.asarray(W_row_out, dtype=np.float32),
        b_row_out=np.asarray(b_row_out, dtype=np.float32),
        W_col_out=np.asarray(W_col_out, dtype=np.float32),
        b_col_out=np.asarray(b_col_out, dtype=np.float32),
    )
    if not (args["row_attention_mask"].all() and args["column_attention_mask"].all()):
        return _numpy_fallback(**args)

    from concourse.bass_utils import run_bass_kernel_spmd

    nc = _get_module()

    colT = np.ascontiguousarray(args["column_embeddings"].T)
    w_row_aug = np.ascontiguousarray(
        np.concatenate([args["W_row_out"].T, args["b_row_out"][None, :]], axis=0)
    )
    w_col_aug = np.ascontiguousarray(
        np.concatenate([args["W_col_out"].T, args["b_col_out"][None, :]], axis=0)
    )
    shared = {
        "colT": colT,
        "wt_rc_q": np.ascontiguousarray(args["W_rc_q"].T),
        "wt_rc_k": np.ascontiguousarray(args["W_rc_k"].T),
        "wt_rc_v": np.ascontiguousarray(args["W_rc_v"].T),
        "wt_cr_q": np.ascontiguousarray(args["W_cr_q"].T),
        "wt_cr_k": np.ascontiguousarray(args["W_cr_k"].T),
        "wt_cr_v": np.ascontiguousarray(args["W_cr_v"].T),
        "w_row_aug": w_row_aug,
        "w_col_aug": w_col_aug,
    }
    in_maps = []
    for c in range(_NCORES):
        rT = np.ascontiguousarray(
            args["row_embeddings"][c * _NB : (c + 1) * _NB].transpose(0, 2, 1)
        )
        in_maps.append({"rowT": rT, **shared})

    res = run_bass_kernel_spmd(nc, in_maps, core_ids=list(range(_NCORES)))

    fused_row = np.empty((_B, _F, _D), np.float32)
    fused_col = np.empty((_B, _F, _D), np.float32)
    r2c_w = np.empty((_B, _F, _F), np.float32)
    c2r_w = np.empty((_B, _F, _F), np.float32)
    for c in range(_NCORES):
        out = res.results[c]
        sl = slice(c * _NB, (c + 1) * _NB)
        fused_row[sl] = out["fusedT_row"].transpose(0, 2, 1)
        fused_col[sl] = out["fusedT_col"].transpose(0, 2, 1)
        # device accumulates sum over heads; apply the 1/H mean factor here
        r2c_w[sl] = out["meanwT_r2c"].transpose(0, 2, 1) * (1.0 / _H)
        c2r_w[sl] = out["meanwT_c2r"].transpose(0, 2, 1) * (1.0 / _H)
    return (fused_row, fused_col, r2c_w, c2r_w)
